# revision 1
# baseline (speedup 1.0000x reference)
"""Trainium2 Bass kernel: single-head causal self-attention.

Reference computation (per batch b):
    Q = x @ Wq ; K = x @ Wk ; V = x @ Wv          (x: [S, D])
    S_sc = Q @ K^T / sqrt(D), causal masked
    out  = softmax(S_sc) @ V

Sharding: 8 cores, 4 batches -> core c handles batch b = c//2 and query
half h = c%2 (1024 query rows), with full K/V for that batch computed
on-core (redundantly for the pair). Uniform SPMD program; per-core
behavior comes only from input data (xqT slice + global-q-index vector
used to build the causal mask on device).

Layout strategy (all fp32):
  - Host passes x[b]^T so the contraction dim (d_in) lands on partitions.
  - K^T [d, S] stays resident in SBUF; V [S, d] is staged to DRAM during
    the projection phase and streamed back per q-strip.
  - Scores are computed TRANSPOSED: S^T[k, q] = sum_d K^T[d,k] * Q^T[d,q],
    so softmax's k-reduction lands on the partition dim; the sum is done
    with an extra N=1 matmul against a ones vector (riding the same
    stationary P^T tile as the P@V matmuls), avoiding any P transposes.
  - No max-subtraction in softmax: scores ~ N(0,1), exp is safe in fp32.
  - Causal mask built on device: mask[k,q] = (q_global >= k_global),
    multiplied into exp(S^T) (multiplicative 0/1 mask after exp).
"""

import sys

try:
    import concourse.bass as bass  # noqa: F401
except ImportError:
    sys.path.insert(0, "/opt/trn_rl_repo")

import numpy as np

import concourse.bass as bass
import concourse.tile as tile
from concourse import bacc, mybir
from concourse.bass_utils import run_bass_kernel_spmd

B, S, D = 4, 2048, 1024
NQ = 1024  # query rows per core
NK = 2048  # keys per core
P = 128
DT = D // P  # 8 d tiles
KT = NK // P  # 16 k tiles
W = 256  # q-strip width
NSTRIP = NQ // W  # 4 strips
F32 = mybir.dt.float32
SCALE = 1.0 / np.sqrt(np.float32(D))  # 0.03125

_NC_CACHE = {}


def build_nc(mm_dt=F32):
    nc = bacc.Bacc(None, target_bir_lowering=False)
    xkvT = nc.dram_tensor("xkvT", [D, NK], mm_dt, kind="ExternalInput")
    xqT = nc.dram_tensor("xqT", [D, NQ], mm_dt, kind="ExternalInput")
    qg = nc.dram_tensor("qg", [NQ], F32, kind="ExternalInput")
    wq_d = nc.dram_tensor("Wq", [D, D], mm_dt, kind="ExternalInput")
    wk_d = nc.dram_tensor("Wk", [D, D], mm_dt, kind="ExternalInput")
    wv_d = nc.dram_tensor("Wv", [D, D], mm_dt, kind="ExternalInput")
    out_d = nc.dram_tensor("out", [NQ, D], F32, kind="ExternalOutput")
    vdram = nc.dram_tensor("vscratch", [NK, D], mm_dt)

    with tile.TileContext(nc) as tc:
        with (
            tc.tile_pool(name="persist", bufs=1) as persist,
            tc.tile_pool(name="misc", bufs=1) as misc,
        ):
            # Persistent K^T [d, NK] (8 partition-tiles)
            kT = persist.tile([P, DT, NK], mm_dt, tag="kT")

            # Small constants: ones columns (2 wide: fp32r matmuls need an
            # even moving dim), k-index vectors for the mask
            ones_f = misc.tile([P, 2], F32, tag="ones_f")
            nc.vector.memset(ones_f, 1.0)
            ones = misc.tile([P, 2], mm_dt, tag="ones")
            nc.vector.tensor_copy(ones, ones_f)
            pvec_i = misc.tile([P, 1], mybir.dt.int32, tag="pvec_i")
            nc.gpsimd.iota(pvec_i, pattern=[[0, 1]], base=0, channel_multiplier=1)
            pvec = misc.tile([P, 1], F32, tag="pvec")
            nc.vector.tensor_copy(pvec, pvec_i)
            kvecf = misc.tile([P, KT], F32, tag="kvecf")
            for kt in range(KT):
                nc.vector.tensor_scalar_add(kvecf[:, kt : kt + 1], pvec, float(kt * P))

            # ------------- Phase 1: K^T (SBUF) and V (-> DRAM) -------------
            with (
                tc.tile_pool(name="wkv", bufs=1) as wkvp,
                tc.tile_pool(name="xin", bufs=2) as xinp,
                tc.tile_pool(name="vstage", bufs=2) as vsp,
                tc.tile_pool(name="ps1", bufs=4, space="PSUM") as ps1,
            ):
                wk = wkvp.tile([P, DT, D], mm_dt, tag="wk")
                wv = wkvp.tile([P, DT, D], mm_dt, tag="wv")
                wk_t = wk_d.rearrange("(a p) o -> p a o", p=P)
                wv_t = wv_d.rearrange("(a p) o -> p a o", p=P)
                # all of wk first: the first K^T group accumulates over all
                # 8 d_in tiles, so wk's arrival gates PE start; wv is not
                # needed until the V section
                for di in range(DT):
                    nc.scalar.dma_start(wk[:, di, :], wk_t[:, di, :])
                for di in range(DT):
                    nc.scalar.dma_start(wv[:, di, :], wv_t[:, di, :])
                xkvT_t = xkvT.rearrange("(a p) s -> p a s", p=P)

                for qr in range(4):  # quarters of the key sequence
                    s0 = qr * 512
                    xin = xinp.tile([P, DT, 512], mm_dt, tag="xin")
                    nc.sync.dma_start(xin, xkvT_t[:, :, s0 : s0 + 512])
                    # K^T tiles: out[d_out, s] accumulated over d_in
                    for do in range(DT):
                        ps = ps1.tile([P, 512], F32, tag="ps1")
                        for di in range(DT):
                            nc.tensor.matmul(
                                ps,
                                wk[:, di, do * P : (do + 1) * P],
                                xin[:, di, :],
                                start=(di == 0),
                                stop=(di == DT - 1),
                            )
                        nc.vector.tensor_copy(kT[:, do, s0 : s0 + 512], ps)
                    # V tiles: out[s, d_out] accumulated over d_in -> DRAM
                    for st in range(4):
                        gst = qr * 4 + st
                        vstage = vsp.tile([P, D], mm_dt, tag="vstage")
                        for dh in range(2):
                            ps = ps1.tile([P, 512], F32, tag="ps1")
                            for di in range(DT):
                                nc.tensor.matmul(
                                    ps,
                                    xin[:, di, st * P : (st + 1) * P],
                                    wv[:, di, dh * 512 : (dh + 1) * 512],
                                    start=(di == 0),
                                    stop=(di == DT - 1),
                                )
                            nc.vector.tensor_copy(vstage[:, dh * 512 : (dh + 1) * 512], ps)
                        nc.sync.dma_start(vdram[gst * P : (gst + 1) * P, :], vstage)

            # ---------------- Phase 2: per-q-strip attention ----------------
            with (
                tc.tile_pool(name="wqp", bufs=1) as wqp,
                tc.tile_pool(name="strip", bufs=1) as strip,
                tc.tile_pool(name="vs2", bufs=4) as vs2,
                tc.tile_pool(name="sm", bufs=4) as sm,
                tc.tile_pool(name="outp", bufs=2) as outp,
                tc.tile_pool(name="ps2", bufs=2, space="PSUM") as ps2p,
                tc.tile_pool(name="psc", bufs=2, space="PSUM") as pscp,
                tc.tile_pool(name="psl", bufs=2, space="PSUM") as pslp,
            ):
                wq = wqp.tile([P, DT, D], mm_dt, tag="wq")
                wq_t = wq_d.rearrange("(a p) o -> p a o", p=P)
                for di in range(DT):
                    nc.scalar.dma_start(wq[:, di, :], wq_t[:, di, :])
                xqT_t = xqT.rearrange("(a p) s -> p a s", p=P)

                for qs in range(NSTRIP):
                    q0 = qs * W
                    qx = strip.tile([P, DT, W], mm_dt, tag="qx", bufs=2)
                    nc.scalar.dma_start(qx, xqT_t[:, :, q0 : q0 + W])
                    # Q^T strip [d, W]
                    qT = strip.tile([P, DT, W], mm_dt, tag="qT")
                    for do in range(DT):
                        ps = ps2p.tile([P, W], F32, tag="ps2")
                        for di in range(DT):
                            nc.tensor.matmul(
                                ps,
                                wq[:, di, do * P : (do + 1) * P],
                                qx[:, di, :],
                                start=(di == 0),
                                stop=(di == DT - 1),
                            )
                        nc.vector.tensor_copy(qT[:, do, :], ps)

                    # broadcast global q indices for this strip to all partitions
                    qgrid = sm.tile([P, W], F32, tag="qgrid")
                    qg_sl = qg[q0 : q0 + W]
                    nc.gpsimd.dma_start(
                        qgrid,
                        bass.AP(
                            tensor=qg_sl.tensor,
                            offset=qg_sl.offset,
                            ap=[[0, P]] + list(qg_sl.ap),
                        ),
                    )

                    # S^T strip -> exp -> mask -> P^T strip.
                    # Causal interleave: this strip holds global q-tiles
                    # 2j+h for j in {2qs, 2qs+1}, so k-tiles >= ext_kt are
                    # fully masked and skipped at compile time.
                    ext_kt = 4 * (qs + 1)
                    pT = strip.tile([P, KT, W], mm_dt, tag="pT")
                    for kt in range(ext_kt):
                        ps = ps2p.tile([P, W], F32, tag="ps2")
                        for di in range(DT):
                            nc.tensor.matmul(
                                ps,
                                kT[:, di, kt * P : (kt + 1) * P],
                                qT[:, di, :],
                                start=(di == 0),
                                stop=(di == DT - 1),
                            )
                        et = sm.tile([P, W], F32, tag="et")
                        nc.scalar.activation(
                            et, ps, mybir.ActivationFunctionType.Exp, scale=float(SCALE)
                        )
                        mt = sm.tile([P, W], F32, tag="mt")
                        nc.vector.tensor_scalar(
                            mt,
                            qgrid,
                            kvecf[:, kt : kt + 1],
                            None,
                            op0=mybir.AluOpType.is_ge,
                        )
                        nc.vector.tensor_mul(pT[:, kt, :], et, mt)

                    # context = P^T.T @ V (V streamed from DRAM, kt-outer),
                    # row-sums l via ones column riding the same stationary P^T
                    ncq = W // P
                    cps = [
                        pscp.tile([P, D], F32, tag="psc", name=f"cps{i}")
                        for i in range(ncq)
                    ]
                    lps = [
                        pslp.tile([P, 2], F32, tag="psl", name=f"lps{i}")
                        for i in range(ncq)
                    ]
                    for kt in range(ext_kt):
                        vt_t = vs2.tile([P, D], mm_dt, tag="vstrip")
                        nc.sync.dma_start(vt_t, vdram[kt * P : (kt + 1) * P, :])
                        for qt in range(ncq):
                            ej = 2 * (qs * ncq + qt) + 2  # this position's extent
                            if kt >= ej:
                                continue
                            lhs = pT[:, kt, qt * P : (qt + 1) * P]
                            nc.tensor.matmul(
                                cps[qt][:, 0:512],
                                lhs,
                                vt_t[:, 0:512],
                                start=(kt == 0),
                                stop=(kt == ej - 1),
                            )
                            nc.tensor.matmul(
                                cps[qt][:, 512:1024],
                                lhs,
                                vt_t[:, 512:1024],
                                start=(kt == 0),
                                stop=(kt == ej - 1),
                            )
                            nc.tensor.matmul(
                                lps[qt],
                                lhs,
                                ones,
                                start=(kt == 0),
                                stop=(kt == ej - 1),
                            )
                    for qt in range(ncq):
                        qrow = q0 + qt * P
                        rt = sm.tile([P, 1], F32, tag="rt")
                        nc.vector.reciprocal(rt, lps[qt][:, 0:1])
                        ot = outp.tile([P, D], F32, tag="ot")
                        nc.vector.tensor_scalar_mul(ot, cps[qt], rt)
                        nc.sync.dma_start(out_d[qrow : qrow + P, :], ot)
    nc.compile()
    return nc


def _get_nc(key="f32"):
    if key not in _NC_CACHE:
        _NC_CACHE[key] = build_nc(F32 if key == "f32" else mybir.dt.float32r)
    return _NC_CACHE[key]


def _qsel(h):
    """Query rows for core-half h: global q-tiles h, 2+h, ..., 14+h.

    Position j's tile 2j+h needs only k < (2j+h+1)*128, letting the kernel
    skip fully-masked k-tiles at compile time with a core-uniform program."""
    tiles = np.arange(8) * 2 + h
    return (tiles[:, None] * P + np.arange(P)[None, :]).reshape(-1)


def make_in_maps(x, Wq, Wk, Wv):
    x = np.asarray(x, dtype=np.float32)
    Wq = np.ascontiguousarray(np.asarray(Wq, dtype=np.float32))
    Wk = np.ascontiguousarray(np.asarray(Wk, dtype=np.float32))
    Wv = np.ascontiguousarray(np.asarray(Wv, dtype=np.float32))
    in_maps = []
    for c in range(8):
        b, h = c // 2, c % 2
        qsel = _qsel(h)
        in_maps.append(
            {
                "xkvT": np.ascontiguousarray(x[b].T),
                "xqT": np.ascontiguousarray(x[b][qsel].T),
                "qg": qsel.astype(np.float32),
                "Wq": Wq,
                "Wk": Wk,
                "Wv": Wv,
            }
        )
    return in_maps


def kernel(x, Wq, Wk, Wv, _trace=False, _nc_key="f32r"):
    nc = _get_nc(_nc_key)
    in_maps = make_in_maps(x, Wq, Wk, Wv)
    res = run_bass_kernel_spmd(nc, in_maps, core_ids=list(range(8)), trace=_trace)
    out = np.empty((B, S, D), dtype=np.float32)
    for c in range(8):
        b, h = c // 2, c % 2
        out[b, _qsel(h), :] = res.results[c]["out"]
    if _trace:
        kernel.last_results = res
    return out



# revision 3
# speedup vs baseline: 1.2522x; 1.2522x over previous
"""Trainium2 Bass kernel: single-head causal self-attention (fp16 + pairwise K/V dedup).

Reference computation (per batch b):
    Q = x @ Wq ; K = x @ Wk ; V = x @ Wv          (x: [S, D])
    S_sc = Q @ K^T / sqrt(D), causal masked
    out  = softmax(S_sc) @ V

Sharding: 8 cores, 4 batches -> core c handles batch b = c//2 and query
half h = c%2 (1024 interleaved query rows). K/V projections are
DEDUPLICATED across the pair: core h computes K^T and V only for keys
[h*1024, (h+1)*1024), then the halves are exchanged with pairwise
AllGather collectives ([[0,1],[2,3],[4,5],[6,7]]) through DRAM bounce
buffers. The program stays core-uniform: gathered outputs are written
back over the FULL K^T/V SBUF tiles (own half is overwritten with
identical data), so no instruction depends on h.

Layout strategy:
  - All matmul operands are fp16 (host-converted); PSUM/softmax math fp32.
  - Host passes x^T slices so the contraction dim (d_in) is on partitions.
  - K^T [d, S] and V [S, d] fully SBUF-resident for the attention phase.
  - Scores computed TRANSPOSED: S^T[k, q] = sum_d K^T[d,k]*Q^T[d,q]; the
    softmax k-reduction is done with an N=2 ones-matmul riding the same
    stationary P^T tiles as the P@V matmuls.
  - No max-subtraction in softmax (scores ~ N(0,1), exp safe in fp32).
  - Causal mask: only the 4 diagonal-crossing k-tiles per strip get the
    (q_global >= k_global) multiplicative mask; earlier k-tiles are fully
    kept and exp() writes straight into P^T. k-tiles beyond each
    position's extent are skipped at compile time (core-uniform bounds).
  - V is gathered in two quarter-collectives so early PV k-tiles are
    available before the full projection finishes.
"""

import sys

try:
    import concourse.bass as bass  # noqa: F401
except ImportError:
    sys.path.insert(0, "/opt/trn_rl_repo")

import numpy as np

import concourse.bass as bass
import concourse.tile as tile
from concourse import bacc, mybir
from concourse.bass_utils import run_bass_kernel_spmd

B, S, D = 4, 2048, 1024
NQ = 1024  # query rows per core
HK = 1024  # keys projected per core (half of S)
P = 128
DT = D // P  # 8 d tiles
KT = S // P  # 16 k tiles
W = 256  # q-strip width
NSTRIP = NQ // W  # 4 strips
F32 = mybir.dt.float32
F16 = mybir.dt.float16
SCALE = 1.0 / np.sqrt(np.float32(D))  # 0.03125
GROUPS = [[0, 1], [2, 3], [4, 5], [6, 7]]

_NC_CACHE = {}


def build_nc():
    nc = bacc.Bacc(None, target_bir_lowering=False, num_devices=8)
    xkvT = nc.dram_tensor("xkvT", [D, HK], F16, kind="ExternalInput")
    xqT = nc.dram_tensor("xqT", [D, NQ], F16, kind="ExternalInput")
    qg = nc.dram_tensor("qg", [NQ], F32, kind="ExternalInput")
    wq_d = nc.dram_tensor("Wq", [D, D], F16, kind="ExternalInput")
    wk_d = nc.dram_tensor("Wk", [D, D], F16, kind="ExternalInput")
    wv_d = nc.dram_tensor("Wv", [D, D], F16, kind="ExternalInput")
    out_d = nc.dram_tensor("out", [NQ, D], F32, kind="ExternalOutput")
    # collective bounce buffers (HBM; Shared outs unsupported for 2-rank groups)
    kag_in = nc.dram_tensor("kag_in", [D, HK], F16)
    kag_out = nc.dram_tensor("kag_out", [2 * D, HK], F16)
    vag_in = [nc.dram_tensor(f"vag_in{i}", [512, D], F16) for i in range(2)]
    vag_out = [nc.dram_tensor(f"vag_out{i}", [1024, D], F16) for i in range(2)]

    with tile.TileContext(nc) as tc:
        with (
            tc.tile_pool(name="persist", bufs=1) as persist,
            tc.tile_pool(name="misc", bufs=1) as misc,
        ):
            # Persistent attention operands
            kT = persist.tile([P, DT, S], F16, tag="kT")  # K^T, full
            vsb = persist.tile([P, KT, D], F16, tag="vsb")  # V, full
            xq = persist.tile([P, DT, NQ], F16, tag="xq")  # x^T for own queries
            wq = persist.tile([P, DT, D], F16, tag="wq")
            qgrid = persist.tile([P, NSTRIP, W], F32, tag="qgrid")

            # Constants: ones column (N=2), per-partition k index vectors
            ones_f = misc.tile([P, 2], F32, tag="ones_f")
            nc.vector.memset(ones_f, 1.0)
            ones = misc.tile([P, 2], F16, tag="ones")
            nc.vector.tensor_copy(ones, ones_f)
            pvec_i = misc.tile([P, 1], mybir.dt.int32, tag="pvec_i")
            nc.gpsimd.iota(pvec_i, pattern=[[0, 1]], base=0, channel_multiplier=1)
            pvec = misc.tile([P, 1], F32, tag="pvec")
            nc.vector.tensor_copy(pvec, pvec_i)
            kvecf = misc.tile([P, KT], F32, tag="kvecf")
            for kt in range(KT):
                nc.vector.tensor_scalar_add(kvecf[:, kt : kt + 1], pvec, float(kt * P))

            # Broadcast global q indices for each strip to all partitions
            for qs in range(NSTRIP):
                qg_sl = qg[qs * W : (qs + 1) * W]
                nc.gpsimd.dma_start(
                    qgrid[:, qs, :],
                    bass.AP(
                        tensor=qg_sl.tensor,
                        offset=qg_sl.offset,
                        ap=[[0, P]] + list(qg_sl.ap),
                    ),
                )

            # ---- Phase 1: own-half K^T and V -> bounce DRAM -> AllGather ----
            with (
                tc.tile_pool(name="wkv", bufs=1) as wkvp,
                tc.tile_pool(name="stg", bufs=4) as stgp,
                tc.tile_pool(name="ps1", bufs=4, space="PSUM") as ps1,
            ):
                wk = wkvp.tile([P, DT, D], F16, tag="wk")
                wv = wkvp.tile([P, DT, D], F16, tag="wv")
                xin = wkvp.tile([P, DT, HK], F16, tag="xin")
                wk_t = wk_d.rearrange("(a p) o -> p a o", p=P)
                wv_t = wv_d.rearrange("(a p) o -> p a o", p=P)
                xkvT_t = xkvT.rearrange("(a p) s -> p a s", p=P)
                xqT_t = xqT.rearrange("(a p) s -> p a s", p=P)
                wq_t = wq_d.rearrange("(a p) o -> p a o", p=P)
                # interleave wk/xin arrival per d-tile so PE starts early
                for di in range(DT):
                    nc.scalar.dma_start(wk[:, di, :], wk_t[:, di, :])
                    nc.sync.dma_start(xin[:, di, :], xkvT_t[:, di, :])
                for di in range(DT):
                    nc.scalar.dma_start(wv[:, di, :], wv_t[:, di, :])
                # prefetch phase-2 operands now; DMA engines are otherwise idle
                nc.scalar.dma_start(wq, wq_t)
                nc.sync.dma_start(xq, xqT_t)

                kag_in_v = kag_in.rearrange("(a p) s -> p a s", p=P)
                # K^T own half: out[d_out, s_own] accumulated over d_in
                for ch in range(2):
                    for do in range(DT):
                        ps = ps1.tile([P, 512], F32, tag="ps1")
                        for di in range(DT):
                            nc.tensor.matmul(
                                ps,
                                wk[:, di, do * P : (do + 1) * P],
                                xin[:, di, ch * 512 : (ch + 1) * 512],
                                start=(di == 0),
                                stop=(di == DT - 1),
                            )
                        kst = stgp.tile([P, 512], F16, tag="kst")
                        nc.vector.tensor_copy(kst, ps)
                        nc.sync.dma_start(
                            kag_in_v[:, do, ch * 512 : (ch + 1) * 512], kst
                        )
                nc.gpsimd.collective_compute(
                    "AllGather",
                    mybir.AluOpType.bypass,
                    replica_groups=GROUPS,
                    ins=[kag_in[:, :].opt()],
                    outs=[kag_out[:, :].opt()],
                )
                # full overwrite keeps the program core-uniform
                for half in range(2):
                    src = kag_out[half * D : (half + 1) * D, :].rearrange(
                        "(a p) s -> p a s", p=P
                    )
                    nc.scalar.dma_start(kT[:, :, half * HK : (half + 1) * HK], src)

                # V own half, in two quarters; each quarter AllGathers as soon
                # as its 8 stage DMAs land
                for sq in range(2):
                    vag_in_v = vag_in[sq].rearrange("(st p) d -> p st d", p=P)
                    for st in range(4):
                        for dh in range(2):
                            ps = ps1.tile([P, 512], F32, tag="ps1")
                            for di in range(DT):
                                nc.tensor.matmul(
                                    ps,
                                    xin[
                                        :,
                                        di,
                                        sq * 512 + st * P : sq * 512 + (st + 1) * P,
                                    ],
                                    wv[:, di, dh * 512 : (dh + 1) * 512],
                                    start=(di == 0),
                                    stop=(di == DT - 1),
                                )
                            vst = stgp.tile([P, 512], F16, tag="vst")
                            nc.vector.tensor_copy(vst, ps)
                            nc.sync.dma_start(
                                vag_in_v[:, st, dh * 512 : (dh + 1) * 512], vst
                            )
                    nc.gpsimd.collective_compute(
                        "AllGather",
                        mybir.AluOpType.bypass,
                        replica_groups=GROUPS,
                        ins=[vag_in[sq][:, :].opt()],
                        outs=[vag_out[sq][:, :].opt()],
                    )
                    # shard 0 -> global k-tiles 4sq..4sq+3, shard 1 -> 8+4sq..
                    for half in range(2):
                        src = vag_out[sq][half * 512 : (half + 1) * 512, :].rearrange(
                            "(st p) d -> p st d", p=P
                        )
                        t0 = half * 8 + sq * 4
                        nc.scalar.dma_start(vsb[:, t0 : t0 + 4, :], src)

            # ---------------- Phase 2: per-q-strip attention ----------------
            with (
                tc.tile_pool(name="strip", bufs=2) as strip,
                tc.tile_pool(name="sm", bufs=4) as sm,
                tc.tile_pool(name="outp", bufs=2) as outp,
                tc.tile_pool(name="ps2", bufs=2, space="PSUM") as ps2p,
                tc.tile_pool(name="psc", bufs=2, space="PSUM") as pscp,
                tc.tile_pool(name="psl", bufs=2, space="PSUM") as pslp,
            ):
                for qs in range(NSTRIP):
                    q0 = qs * W
                    # Q^T strip [d, W]
                    qT = strip.tile([P, DT, W], F16, tag="qT")
                    for do in range(DT):
                        ps = ps2p.tile([P, W], F32, tag="ps2")
                        for di in range(DT):
                            nc.tensor.matmul(
                                ps,
                                wq[:, di, do * P : (do + 1) * P],
                                xq[:, di, q0 : q0 + W],
                                start=(di == 0),
                                stop=(di == DT - 1),
                            )
                        nc.vector.tensor_copy(qT[:, do, :], ps)

                    # S^T strip -> exp -> (mask) -> P^T strip.
                    # Strip holds global q-tiles 4qs+h and 4qs+2+h, so k-tiles
                    # >= ext_kt are fully masked for both halves and skipped;
                    # k-tiles < 4qs are fully kept for both halves.
                    ext_kt = 4 * (qs + 1)
                    pT = strip.tile([P, KT, W], F16, tag="pT")
                    for kt in range(ext_kt):
                        ps = ps2p.tile([P, W], F32, tag="ps2")
                        for di in range(DT):
                            nc.tensor.matmul(
                                ps,
                                kT[:, di, kt * P : (kt + 1) * P],
                                qT[:, di, :],
                                start=(di == 0),
                                stop=(di == DT - 1),
                            )
                        if kt < 4 * qs:
                            nc.scalar.activation(
                                pT[:, kt, :],
                                ps,
                                mybir.ActivationFunctionType.Exp,
                                scale=float(SCALE),
                            )
                        else:
                            et = sm.tile([P, W], F32, tag="et")
                            nc.scalar.activation(
                                et, ps, mybir.ActivationFunctionType.Exp,
                                scale=float(SCALE),
                            )
                            mt = sm.tile([P, W], F32, tag="mt")
                            nc.vector.tensor_scalar(
                                mt,
                                qgrid[:, qs, :],
                                kvecf[:, kt : kt + 1],
                                None,
                                op0=mybir.AluOpType.is_ge,
                            )
                            nc.vector.tensor_mul(pT[:, kt, :], et, mt)

                    # context = P^T.T @ V (V resident in SBUF), row-sums l via
                    # ones column riding the same stationary P^T
                    ncq = W // P
                    cps = [
                        pscp.tile([P, D], F32, tag="psc", name=f"cps{qs}_{i}")
                        for i in range(ncq)
                    ]
                    lps = [
                        pslp.tile([P, 2], F32, tag="psl", name=f"lps{qs}_{i}")
                        for i in range(ncq)
                    ]
                    for kt in range(ext_kt):
                        for qt in range(ncq):
                            ej = 2 * (qs * ncq + qt) + 2  # this position's extent
                            if kt >= ej:
                                continue
                            lhs = pT[:, kt, qt * P : (qt + 1) * P]
                            nc.tensor.matmul(
                                cps[qt][:, 0:512],
                                lhs,
                                vsb[:, kt, 0:512],
                                start=(kt == 0),
                                stop=(kt == ej - 1),
                            )
                            nc.tensor.matmul(
                                cps[qt][:, 512:1024],
                                lhs,
                                vsb[:, kt, 512:1024],
                                start=(kt == 0),
                                stop=(kt == ej - 1),
                            )
                            nc.tensor.matmul(
                                lps[qt],
                                lhs,
                                ones,
                                start=(kt == 0),
                                stop=(kt == ej - 1),
                            )
                    for qt in range(ncq):
                        qrow = q0 + qt * P
                        rt = sm.tile([P, 1], F32, tag="rt")
                        nc.vector.reciprocal(rt, lps[qt][:, 0:1])
                        ot = outp.tile([P, D], F32, tag="ot")
                        nc.vector.tensor_scalar_mul(ot, cps[qt], rt)
                        nc.sync.dma_start(out_d[qrow : qrow + P, :], ot)
    nc.compile()
    return nc


def _get_nc(key="f16"):
    if "nc" not in _NC_CACHE:
        _NC_CACHE["nc"] = build_nc()
    return _NC_CACHE["nc"]


def _qsel(h):
    """Query rows for core-half h: global q-tiles h, 2+h, ..., 14+h.

    Position j's tile 2j+h needs only k < (2j+h+1)*128, letting the kernel
    skip fully-masked k-tiles at compile time with a core-uniform program."""
    tiles = np.arange(8) * 2 + h
    return (tiles[:, None] * P + np.arange(P)[None, :]).reshape(-1)


def make_in_maps(x, Wq, Wk, Wv):
    x = np.asarray(x, dtype=np.float32)
    Wq16 = np.ascontiguousarray(np.asarray(Wq, dtype=np.float16))
    Wk16 = np.ascontiguousarray(np.asarray(Wk, dtype=np.float16))
    Wv16 = np.ascontiguousarray(np.asarray(Wv, dtype=np.float16))
    in_maps = []
    for c in range(8):
        b, h = c // 2, c % 2
        qsel = _qsel(h)
        in_maps.append(
            {
                "xkvT": np.ascontiguousarray(
                    x[b][h * HK : (h + 1) * HK].T.astype(np.float16)
                ),
                "xqT": np.ascontiguousarray(x[b][qsel].T.astype(np.float16)),
                "qg": qsel.astype(np.float32),
                "Wq": Wq16,
                "Wk": Wk16,
                "Wv": Wv16,
            }
        )
    return in_maps


def kernel(x, Wq, Wk, Wv, _trace=False, _nc_key="f16"):
    nc = _get_nc(_nc_key)
    in_maps = make_in_maps(x, Wq, Wk, Wv)
    res = run_bass_kernel_spmd(nc, in_maps, core_ids=list(range(8)), trace=_trace)
    out = np.empty((B, S, D), dtype=np.float32)
    for c in range(8):
        b, h = c // 2, c % 2
        out[b, _qsel(h), :] = res.results[c]["out"]
    if _trace:
        kernel.last_results = res
    return out
